# revision 16
# baseline (speedup 1.0000x reference)
"""NoisyTopkRouter (MoE routing) Bass kernel for Trainium2, 8-core SPMD.

Problem: x [4,4096,2048] f32, W [128,2048], b [128].
  gate_logits = x @ W.T + b              -> [B,N,128]
  topk_vals, topk_idx = top_k(logits, 8)
  full_gates = scatter(softmax(topk_vals)) -> [B,N,128] dense
Returns (full_gates f32, topk_idx int32).

Strategy (per core, 2048 tokens):
  - 4 groups of 512 tokens. For each group:
    load x natural [128tok, 2048] x4 (DMA), PE-transpose 128x128 blocks into
    xT [128D, 512tok] (PSUM), copy to SBUF (alternating scalar/vector engine),
    f32r matmul with replicated W^T chunks accumulating logits^T [128E, 512tok],
    bias-add on the PSUM->SBUF copy, PE-transpose back to [tok, E],
    then per 128-token tile: vector-engine max (top-8 desc) + max_index,
    exp on scalar engine, fused (logits>=t8)*exp with accumulated denominator,
    reciprocal, normalize on scalar engine. DMA gates + indices out.
"""

import numpy as np

import concourse.bass as bass
import concourse.bacc as bacc
import concourse.tile as tile
import concourse.mybir as mybir
from concourse import bass_utils

B, N, D, E, K = 4, 4096, 2048, 128, 8
NCORES = 8
T_TOTAL = B * N            # 16384 tokens
T_CORE = T_TOTAL // NCORES  # 2048 tokens per core
GROUP = 512                # tokens per group
P = 128                    # partitions
DC = D // P                # 16 contraction chunks
SUB = GROUP // P           # 4 token sub-tiles per group

f32 = mybir.dt.float32
f32r = mybir.dt.float32r
u32 = mybir.dt.uint32
i32 = mybir.dt.int32

AF = mybir.ActivationFunctionType
ALU = mybir.AluOpType


def build_program(t_core=T_CORE, num_devices=NCORES, debug=False):
    """Build + compile the per-core Bass program (SPMD: same program, different data)."""
    nc = bacc.Bacc(
        "TRN2",
        target_bir_lowering=False,
        debug=debug,
        enable_asserts=True,
        num_devices=num_devices,
    )

    ngroups = t_core // GROUP
    assert t_core % GROUP == 0

    x_d = nc.dram_tensor("x", [t_core, D], f32, kind="ExternalInput")
    wt_d = nc.dram_tensor("wt", [D, E], f32, kind="ExternalInput")     # W^T
    b_d = nc.dram_tensor("bias", [E, 1], f32, kind="ExternalInput")
    id_d = nc.dram_tensor("ident", [P, P], f32, kind="ExternalInput")
    gates_d = nc.dram_tensor("gates", [t_core, E], f32, kind="ExternalOutput")
    idx_d = nc.dram_tensor("idx", [t_core, K], i32, kind="ExternalOutput")

    with tile.TileContext(nc) as tc:
        with (
            tc.tile_pool(name="const", bufs=1) as constp,
            tc.tile_pool(name="x", bufs=2 * SUB) as xp,
            tc.tile_pool(name="xt", bufs=3) as xtp,
            tc.tile_pool(name="lt", bufs=2) as ltp,
            tc.tile_pool(name="work", bufs=2 * SUB) as workp,
            tc.tile_pool(name="small", bufs=4 * SUB) as smallp,
            tc.tile_pool(name="out", bufs=2) as outp,
            tc.tile_pool(name="ps_xt", bufs=2, space=bass.MemorySpace.PSUM) as ps_xt,
            tc.tile_pool(name="ps_lt", bufs=2, space=bass.MemorySpace.PSUM) as ps_lt,
            tc.tile_pool(name="ps_lg", bufs=2, space=bass.MemorySpace.PSUM) as ps_lg,
        ):
            # Constants: W^T chunks [p, chunk, e], bias [E,1], identity
            wt_sb = constp.tile([P, DC, E], f32)
            nc.sync.dma_start(wt_sb[:], wt_d.ap().rearrange("(c p) e -> p c e", p=P))
            bias_sb = constp.tile([P, 1], f32)
            nc.sync.dma_start(bias_sb[:], b_d.ap())
            ident_sb = constp.tile([P, P], f32)
            nc.sync.dma_start(ident_sb[:], id_d.ap())

            for g in range(ngroups):
                # --- load x natural: SUB tiles of [128 tok, D] ---
                x_tiles = []
                for t in range(SUB):
                    xt_t = xp.tile([P, D], f32, tag="xtile")
                    row0 = g * GROUP + t * P
                    nc.sync.dma_start(xt_t[:], x_d.ap()[row0 : row0 + P, :])
                    x_tiles.append(xt_t)

                # --- transpose + matmul accumulate logits^T [E, GROUP] ---
                lt_ps = ps_lt.tile([P, GROUP], f32)
                for c in range(DC):
                    xt_ps = ps_xt.tile([P, GROUP], f32, tag="xtps")
                    for t in range(SUB):
                        nc.tensor.matmul(
                            xt_ps[:, t * P : (t + 1) * P],
                            x_tiles[t][:, c * P : (c + 1) * P],
                            ident_sb[:],
                            is_transpose=True,
                            start=(t == 0),
                            stop=(t == SUB - 1),
                        )
                    xt_sb = xtp.tile([P, GROUP], f32, tag="xtsb")
                    if c % 2 == 0:
                        nc.scalar.copy(xt_sb[:], xt_ps[:])
                    else:
                        nc.vector.tensor_copy(xt_sb[:], xt_ps[:])
                    nc.tensor.matmul(
                        lt_ps[:],
                        wt_sb[:, c, :],
                        xt_sb[:],
                        start=(c == 0),
                        stop=(c == DC - 1),
                    )

                # --- bias add on copy-out (per-partition bias; E on partitions) ---
                lt_sb = ltp.tile([P, GROUP], f32)
                nc.scalar.activation(lt_sb[:], lt_ps[:], AF.Identity, bias=bias_sb[:])

                # --- transpose logits^T -> logits [tok, E] ---
                lg_ps = ps_lg.tile([P, SUB, E], f32)
                for t in range(SUB):
                    nc.tensor.matmul(
                        lg_ps[:, t, :],
                        lt_sb[:, t * P : (t + 1) * P],
                        ident_sb[:],
                        is_transpose=True,
                        start=(t == 0),
                        stop=(t == SUB - 1),
                    )

                # --- per 128-token tile: top-8 + softmax + dense scatter ---
                # stage-major emission pipelines the serial per-tile chains
                # across tiles on the two engines
                gates_sb = outp.tile([P, SUB, E], f32, tag="gates")
                idx_sb = outp.tile([P, SUB, K], u32, tag="idx")
                max8s, exps, maskeds, recs = [], [], [], []
                for t in range(SUB):
                    max8 = smallp.tile([P, K], f32, tag=f"max8_{t}")
                    nc.vector.max(max8[:], lg_ps[:, t, :])
                    max8s.append(max8)
                    exp_sb = workp.tile([P, E], f32, tag=f"exp_{t}")
                    nc.scalar.activation(exp_sb[:], lg_ps[:, t, :], AF.Exp)
                    exps.append(exp_sb)
                for t in range(SUB):
                    nc.vector.max_index(idx_sb[:, t, :], max8s[t][:], lg_ps[:, t, :])
                for t in range(SUB):
                    masked = workp.tile([P, E], f32, tag=f"masked_{t}")
                    denom = smallp.tile([P, 1], f32, tag=f"denom_{t}")
                    nc.vector.scalar_tensor_tensor(
                        masked[:],
                        lg_ps[:, t, :],
                        max8s[t][:, K - 1 : K],
                        exps[t][:],
                        op0=ALU.is_ge,
                        op1=ALU.mult,
                        accum_out=denom[:],
                    )
                    rec = smallp.tile([P, 1], f32, tag=f"rec_{t}")
                    nc.vector.reciprocal(rec[:], denom[:])
                    maskeds.append(masked)
                    recs.append(rec)
                for t in range(SUB):
                    nc.scalar.mul(gates_sb[:, t, :], maskeds[t][:], recs[t][:])

                # --- store outputs ---
                row0 = g * GROUP
                nc.sync.dma_start(
                    gates_d.ap()[row0 : row0 + GROUP, :].rearrange(
                        "(t p) e -> p t e", p=P
                    ),
                    gates_sb[:],
                )
                nc.sync.dma_start(
                    idx_d.ap()[row0 : row0 + GROUP, :].rearrange(
                        "(t p) k -> p t k", p=P
                    ),
                    idx_sb[:].bitcast(i32),
                )

    nc.compile()
    return nc


_CACHE = {}


def _get_program():
    if "nc" not in _CACHE:
        _CACHE["nc"] = build_program()
    return _CACHE["nc"]


def make_in_maps(x, W, b):
    x2d = np.ascontiguousarray(np.asarray(x, dtype=np.float32).reshape(T_TOTAL, D))
    wt = np.ascontiguousarray(np.asarray(W, dtype=np.float32).T)  # [D, E]
    bb = np.ascontiguousarray(np.asarray(b, dtype=np.float32).reshape(E, 1))
    ident = np.eye(P, dtype=np.float32)
    in_maps = []
    for c in range(NCORES):
        in_maps.append(
            {
                "x": x2d[c * T_CORE : (c + 1) * T_CORE],
                "wt": wt,
                "bias": bb,
                "ident": ident,
            }
        )
    return in_maps


def kernel(x, W, b):
    nc = _get_program()
    in_maps = make_in_maps(x, W, b)
    res = bass_utils.run_bass_kernel_spmd(nc, in_maps, core_ids=list(range(NCORES)))
    gates = np.concatenate([res.results[c]["gates"] for c in range(NCORES)], axis=0)
    idx = np.concatenate([res.results[c]["idx"] for c in range(NCORES)], axis=0)
    full_gates = gates.reshape(B, N, E).astype(np.float32)
    topk_idx = idx.reshape(B, N, K).astype(np.int32)
    return full_gates, topk_idx


# revision 46
# speedup vs baseline: 1.2962x; 1.2962x over previous
"""NoisyTopkRouter (MoE routing) Bass kernel for Trainium2, 8-core SPMD.

Problem: x [4,4096,2048] f32, W [128,2048], b [128].
  gate_logits = x @ W.T + b              -> [B,N,128]
  topk_vals, topk_idx = top_k(logits, 8)
  full_gates = scatter(softmax(topk_vals)) -> [B,N,128] dense
Returns (full_gates f32, topk_idx int32).

Strategy (per core, 2048 tokens):
  - 4 groups of 512 tokens. For each group:
    load x natural [128tok, 2048] x4 (DMA), PE-transpose 128x128 blocks into
    xT [128D, 512tok] (PSUM), copy to SBUF (alternating scalar/vector engine),
    f32r matmul with replicated W^T chunks accumulating logits^T [128E, 512tok],
    bias-add on the PSUM->SBUF copy, PE-transpose back to [tok, E],
    then per 128-token tile: vector-engine max (top-8 desc) + max_index,
    exp on scalar engine, fused (logits>=t8)*exp with accumulated denominator,
    reciprocal, normalize on scalar engine. DMA gates + indices out.
"""

import numpy as np

import concourse.bass as bass
import concourse.bacc as bacc
import concourse.tile as tile
import concourse.mybir as mybir
from concourse import bass_utils

B, N, D, E, K = 4, 4096, 2048, 128, 8
NCORES = 8
T_TOTAL = B * N            # 16384 tokens
T_CORE = T_TOTAL // NCORES  # 2048 tokens per core
GROUP = 512                # tokens per group
P = 128                    # partitions
DC = D // P                # 16 contraction chunks
SUB = GROUP // P           # 4 token sub-tiles per group

f32 = mybir.dt.float32
f32r = mybir.dt.float32r
u32 = mybir.dt.uint32
i32 = mybir.dt.int32

AF = mybir.ActivationFunctionType
ALU = mybir.AluOpType


def build_program(t_core=T_CORE, num_devices=NCORES, debug=False):
    """Build + compile the per-core Bass program (SPMD: same program, different data)."""
    nc = bacc.Bacc(
        "TRN2",
        target_bir_lowering=False,
        debug=debug,
        enable_asserts=True,
        num_devices=num_devices,
    )

    ngroups = t_core // GROUP
    assert t_core % GROUP == 0

    x_d = nc.dram_tensor("x", [t_core, D], f32, kind="ExternalInput")
    wt_d = nc.dram_tensor("wt", [D, E], f32, kind="ExternalInput")     # W^T
    b_d = nc.dram_tensor("bias", [E, 1], f32, kind="ExternalInput")
    id_d = nc.dram_tensor("ident", [P, P], f32, kind="ExternalInput")
    gates_d = nc.dram_tensor("gates", [t_core, E], f32, kind="ExternalOutput")
    idx_d = nc.dram_tensor("idx", [t_core, K], i32, kind="ExternalOutput")

    with tile.TileContext(nc) as tc:
        with (
            tc.tile_pool(name="const", bufs=1) as constp,
            tc.tile_pool(name="x", bufs=2) as xp,
            tc.tile_pool(name="xt", bufs=3) as xtp,
            tc.tile_pool(name="lt", bufs=2) as ltp,
            tc.tile_pool(name="work", bufs=2 * SUB) as workp,
            tc.tile_pool(name="small", bufs=4 * SUB) as smallp,
            tc.tile_pool(name="out", bufs=2) as outp,
            tc.tile_pool(name="ps_xt", bufs=3, space=bass.MemorySpace.PSUM) as ps_xt,
            tc.tile_pool(name="ps_lt", bufs=2, space=bass.MemorySpace.PSUM) as ps_lt,
            tc.tile_pool(name="ps_lg", bufs=2, space=bass.MemorySpace.PSUM) as ps_lg,
        ):
            # ident first (gates the very first transpose); the 1MB wt load is
            # issued after group 0's first column-block round so it doesn't
            # block the critical first transposes (wt is only needed by the
            # first chunk matmul, ~6us in).
            ident_sb = constp.tile([P, P], f32)
            nc.sync.dma_start(ident_sb[:], id_d.ap())
            wt_sb = constp.tile([P, DC, E], f32)
            bias_sb = constp.tile([P, 1], f32)

            for g in range(ngroups):
                # --- load x natural as column blocks, interleaved so the
                # first chunks' transposes can start after ~1/CB of the load.
                # group 0 uses finer blocks to shorten the pipeline fill ---
                CB = 4
                CW = D // CB
                x_tiles = [[None] * CB for _ in range(SUB)]
                for cb in range(CB):
                    for t in range(SUB):
                        blk = xp.tile(
                            [P, CW], f32, tag=f"xtile_{t}_{cb}", name=f"xblk_{t}_{cb}"
                        )
                        row0 = g * GROUP + t * P
                        nc.sync.dma_start(
                            blk[:], x_d.ap()[row0 : row0 + P, cb * CW : (cb + 1) * CW]
                        )
                        x_tiles[t][cb] = blk
                    if g == 0 and cb == 0:
                        # on the same (sync) ring as x: issues strictly after
                        # the cb0 blocks, so wt's 1MB doesn't delay them
                        nc.sync.dma_start(
                            wt_sb[:], wt_d.ap().rearrange("(c p) e -> p c e", p=P)
                        )
                        nc.sync.dma_start(bias_sb[:], b_d.ap())

                # --- transpose + matmul accumulate logits^T [E, GROUP] ---
                lt_ps = ps_lt.tile([P, GROUP], f32)
                for c in range(DC):
                    xt_ps = ps_xt.tile([P, GROUP], f32, tag="xtps")
                    cb, ci = divmod(c, DC // CB)  # CB is per-group (8 for g=0)
                    for t in range(SUB):
                        nc.tensor.matmul(
                            xt_ps[:, t * P : (t + 1) * P],
                            x_tiles[t][cb][:, ci * P : (ci + 1) * P],
                            ident_sb[:],
                            is_transpose=True,
                            start=(t == 0),
                            stop=(t == SUB - 1),
                        )
                    xt_sb = xtp.tile([P, GROUP], f32, tag="xtsb")
                    if c % 2 == 0:
                        nc.scalar.copy(xt_sb[:], xt_ps[:])
                    else:
                        nc.vector.tensor_copy(xt_sb[:], xt_ps[:])
                    nc.tensor.matmul(
                        lt_ps[:],
                        wt_sb[:, c, :],
                        xt_sb[:],
                        start=(c == 0),
                        stop=(c == DC - 1),
                    )

                # --- bias add on copy-out (per-partition bias; E on partitions) ---
                lt_sb = ltp.tile([P, GROUP], f32)
                nc.scalar.activation(lt_sb[:], lt_ps[:], AF.Identity, bias=bias_sb[:])

                # --- transpose logits^T -> logits [tok, E] ---
                lg_ps = ps_lg.tile([P, SUB, E], f32)
                for t in range(SUB):
                    nc.tensor.matmul(
                        lg_ps[:, t, :],
                        lt_sb[:, t * P : (t + 1) * P],
                        ident_sb[:],
                        is_transpose=True,
                        start=(t == 0),
                        stop=(t == SUB - 1),
                    )

                # --- copy logits PSUM->SBUF once; avoids PSUM bank-access
                # serialization across the topk ops and gets SBUF DVE speeds ---
                lg_sb = workp.tile([P, SUB, E], f32, tag="lg_sb")
                nc.scalar.copy(lg_sb[:], lg_ps[:])

                def lg(t):
                    return lg_sb[:, t, :]

                # --- per 128-token tile: top-8 + softmax + dense scatter ---
                # stage-major emission pipelines the serial per-tile chains
                # across tiles on the two engines
                gates_tiles = [
                    outp.tile([P, E], f32, tag=f"gates_{t}", name=f"gates_{t}")
                    for t in range(SUB)
                ]
                idx_sb = outp.tile([P, SUB, K], u32, tag="idx")
                max8s, exps, maskeds, recs = [], [], [], []
                for t in range(SUB):
                    max8 = smallp.tile([P, K], f32, tag=f"max8_{t}")
                    nc.vector.max(max8[:], lg(t))
                    max8s.append(max8)
                    exp_sb = workp.tile([P, E], f32, tag=f"exp_{t}")
                    nc.scalar.activation(exp_sb[:], lg(t), AF.Exp)
                    exps.append(exp_sb)
                for t in range(SUB):
                    nc.vector.max_index(idx_sb[:, t, :], max8s[t][:], lg(t))
                for t in range(SUB):
                    masked = workp.tile([P, E], f32, tag=f"masked_{t}")
                    denom = smallp.tile([P, 1], f32, tag=f"denom_{t}")
                    nc.vector.scalar_tensor_tensor(
                        masked[:],
                        lg(t),
                        max8s[t][:, K - 1 : K],
                        exps[t][:],
                        op0=ALU.is_ge,
                        op1=ALU.mult,
                        accum_out=denom[:],
                    )
                    rec = smallp.tile([P, 1], f32, tag=f"rec_{t}")
                    nc.vector.reciprocal(rec[:], denom[:])
                    maskeds.append(masked)
                    recs.append(rec)
                # --- store outputs (per-tile gates DMA overlaps later tiles) ---
                row0 = g * GROUP
                for t in range(SUB):
                    nc.scalar.mul(gates_tiles[t][:], maskeds[t][:], recs[t][:])
                    # alternate HWDGE rings so the final stores don't serialize
                    eng = nc.sync if t % 2 == 0 else nc.scalar
                    eng.dma_start(
                        gates_d.ap()[row0 + t * P : row0 + (t + 1) * P, :],
                        gates_tiles[t][:],
                    )
                nc.scalar.dma_start(
                    idx_d.ap()[row0 : row0 + GROUP, :].rearrange(
                        "(t p) k -> p t k", p=P
                    ),
                    idx_sb[:].bitcast(i32),
                )

    nc.compile()
    return nc


_CACHE = {}


def _get_program():
    if "nc" not in _CACHE:
        _CACHE["nc"] = build_program()
    return _CACHE["nc"]


def make_in_maps(x, W, b):
    x2d = np.ascontiguousarray(np.asarray(x, dtype=np.float32).reshape(T_TOTAL, D))
    wt = np.ascontiguousarray(np.asarray(W, dtype=np.float32).T)  # [D, E]
    bb = np.ascontiguousarray(np.asarray(b, dtype=np.float32).reshape(E, 1))
    ident = np.eye(P, dtype=np.float32)
    in_maps = []
    for c in range(NCORES):
        in_maps.append(
            {
                "x": x2d[c * T_CORE : (c + 1) * T_CORE],
                "wt": wt,
                "bias": bb,
                "ident": ident,
            }
        )
    return in_maps


def kernel(x, W, b):
    nc = _get_program()
    in_maps = make_in_maps(x, W, b)
    res = bass_utils.run_bass_kernel_spmd(nc, in_maps, core_ids=list(range(NCORES)))
    gates = np.concatenate([res.results[c]["gates"] for c in range(NCORES)], axis=0)
    idx = np.concatenate([res.results[c]["idx"] for c in range(NCORES)], axis=0)
    full_gates = gates.reshape(B, N, E).astype(np.float32)
    topk_idx = idx.reshape(B, N, K).astype(np.int32)
    return full_gates, topk_idx
